# revision 10
# baseline (speedup 1.0000x reference)
"""Trainium2 Bass kernel for nn_NeuralLongTermMemory (chunked fast-weight scan).

The per-token fast-weight update is a linear recurrence with per-token scalar
coefficients and rank-1 gradient increments, so each 512-token chunk collapses
into dense matmuls (chunked linear-attention form).  8 cores run one uniform
SPMD program: phase 1 (projections+gates) and phase 3 (output proj + RMSNorm +
gate) are token-parallel (128 tokens/core); the chunk-level column side (grads
at chunk-start weights, decay matrices, state update) is replicated on every
core from an AllGather of k/v/gates; each core computes retrieval rows only
for its own 128 tokens (both chunk passes; the wrong-chunk pass is discarded
by a data-driven select mask).
"""
import numpy as np

N_CORES = 8
H = 1024
D = 256
T = 512
S = 1024
LR = 0.1
GS = 0.1
EPS = 1e-6

_CACHE = {}


# ---------------------------------------------------------------- tile patch
def _patch_tile_drain():
    """This walrus build rejects >1 semaphore wait per instruction; split the
    TileContext exit drain's waits across single-wait NOPs."""
    import concourse.mybir as mybir
    import concourse.tile as tile_mod
    from concourse.tile import TileContext

    if getattr(TileContext, "_nltm_patched", False):
        return

    def _drain_and_barrier(self, tick_clock, wait_clock):
        nc = self.nc
        probe = nc.sync.nop(hint="drain_wait_probe", nofuse=True)
        if probe.ins.sync_info is None:
            probe.ins.sync_info = mybir.SyncInfo(on_wait=[], on_update=[])
        wait_clock.add_sem_waits(
            probe.ins, tile_mod.ScopedClock({None: tick_clock.global_clock}))
        waits = list(probe.ins.sync_info.on_wait or [])
        probe.ins.sync_info.on_wait.clear()
        for w in waits:
            nop = nc.sync.nop(hint="drain_wait_split", nofuse=True)
            nop.ins.sync_info = mybir.SyncInfo(on_wait=[w], on_update=[])
        nc.sync.drain()
        nc.all_engine_barrier()
        assert self.sems is not None
        popped = nc._tile_sem_poison_stack.pop()
        assert popped is self._sem_poison
        nc.clear_and_free_semaphores(list(self.sems.allocated().values()))
        nc.all_engine_barrier()

    TileContext._drain_and_barrier = _drain_and_barrier
    TileContext._nltm_patched = True


def _split_excess_waits(nc, lim=1):
    import concourse.mybir as mybir
    for f in nc.m.functions:
        for bb in f.blocks:
            new_insts = []
            for ins in bb.instructions:
                si = ins.sync_info
                waits = list(si.on_wait) if (si and si.on_wait) else []
                if len(waits) > lim:
                    keep, extra = waits[:lim], waits[lim:]
                    for j in range(0, len(extra), lim):
                        nop = mybir.InstNoOp(
                            name=f"{ins.name}-ws{j}",
                            engine=ins.engine,
                            text_hint="waitsplit",
                            bass_nofuse=True,
                            sync_info=mybir.SyncInfo(
                                on_wait=list(extra[j:j + lim]), on_update=[]),
                        )
                        nc.register_instruction(nop, overwrite=True)
                        new_insts.append(nop)
                    si.on_wait.clear()
                    si.on_wait.extend(keep)
                new_insts.append(ins)
            bb.instructions.clear()
            bb.instructions.extend(new_insts)


# ---------------------------------------------------------------- program
def _build_program(debug=False):
    import contextlib

    import concourse.bass as bass
    import concourse.mybir as mybir
    import concourse.tile as tile

    _patch_tile_drain()

    f32 = mybir.dt.float32
    AF = mybir.ActivationFunctionType
    OP = mybir.AluOpType
    X = mybir.AxisListType.X

    nc = bass.Bass("TRN2")

    def inp(name, shape):
        return nc.dram_tensor(name, shape, f32, kind="ExternalInput")

    hidown = inp("hidown", (128, H))
    hidhalo = inp("hidhalo", (3, H))
    mypos = inp("mypos", (1, 128))
    maskvec = inp("maskvec", (1, 8))
    mysel = inp("mysel", (128, 2))
    wcat = inp("wcat", (H, 1536))
    bcat = inp("bcat", (1, 1536))
    convw = inp("convw", (128, 24))
    convb = inp("convb", (128, 6))
    w1t0 = inp("w1t0", (D, D))
    w2_0 = inp("w2_0", (D, D))
    w2t0 = inp("w2t0", (D, D))
    wot = inp("wot", (D, H))
    gwt = inp("gwt", (H, H))
    ngrow = inp("ngrow", (1, H))
    gbrow = inp("gbrow", (1, H))
    tril = inp("tril", (128, 128))
    ident = inp("ident", (128, 128))
    poscol = inp("poscol", (128, 4))

    outp = nc.dram_tensor("outp", (128, H), f32, kind="ExternalOutput")
    dbg = {}
    if debug:
        def dbgout(name, shape):
            dbg[name] = nc.dram_tensor(name, shape, f32, kind="ExternalOutput")
            return dbg[name]
        dbg_kvq = dbgout("dbg_kvq", (128, 3, 2, 128))
        dbg_gates = dbgout("dbg_gates", (128, 3))
        dbg_cs = dbgout("dbg_cs", (128, 4, 2))
        dbg_dzpp = dbgout("dbg_dzpp", (128, 4, 256, 2))
        dbg_rpp = dbgout("dbg_rpp", (128, 4, 256, 2))
        dbg_st = dbgout("dbg_st", (128, 2, 256, 6))
        dbg_y2 = dbgout("dbg_y2", (128, 256, 2))
        dbg_ret = dbgout("dbg_ret", (128, 2, 128))

    agin = nc.dram_tensor("agin", (128, 515), f32, kind="Internal")
    agout = nc.dram_tensor("agout", (128 * N_CORES, 515), f32, kind="Internal",
                           addr_space="Shared")

    with tile.TileContext(nc) as tc:
        ctx = contextlib.ExitStack()
        with ctx:
            P_const = ctx.enter_context(tc.tile_pool(name="constp", bufs=1))
            P_keep = ctx.enter_context(tc.tile_pool(name="keepp", bufs=1))
            P_cols = ctx.enter_context(tc.tile_pool(name="colsp", bufs=2))
            P_rows = ctx.enter_context(tc.tile_pool(name="rowsp", bufs=1))
            PS_A = ctx.enter_context(tc.tile_pool(name="psa", bufs=5,
                                                  space="PSUM"))
            PS_B = ctx.enter_context(tc.tile_pool(name="psb", bufs=1,
                                                  space="PSUM"))

            MM = nc.tensor.matmul
            ACT = nc.scalar.activation
            V = nc.vector

            def psA(p, name):
                # all PS_A tiles share one 2KB/partition slot set
                return PS_A.tile([p, 512], f32, tag="A", name=name)

            def psB(shape, name):
                return PS_B.tile(shape, f32, tag="B", name=name)

            def cT(pool, shape, tag, bufs=None):
                return pool.tile(shape, f32, tag=tag, name=tag, bufs=bufs)

            # ---------------- constants ----------------
            ones2 = cT(P_const, [128, 128], "ones2")
            V.memset(ones2, 1.0)
            ones_row512 = cT(P_const, [1, T], "onesr")
            V.memset(ones_row512, 1.0)
            sb_ident = cT(P_const, [128, 128], "ident")
            nc.sync.dma_start(sb_ident, ident[:])
            sb_tril = cT(P_const, [128, 128], "tril")
            nc.sync.dma_start(sb_tril, tril[:])
            sb_poscol = cT(P_const, [128, 4], "poscol")
            nc.sync.dma_start(sb_poscol, poscol[:])
            sb_convw = cT(P_const, [128, 24], "convw")
            nc.sync.dma_start(sb_convw, convw[:])
            sb_convb = cT(P_const, [128, 6], "convb")
            nc.sync.dma_start(sb_convb, convb[:])
            sb_mysel = cT(P_const, [128, 2], "mysel")
            nc.sync.dma_start(sb_mysel, mysel[:])
            sb_maskvec = cT(P_const, [1, 8], "maskvec")
            nc.sync.dma_start(sb_maskvec, maskvec[:])
            sb_mypos = cT(P_const, [1, 128], "mypos")
            nc.sync.dma_start(sb_mypos, mypos[:])
            sb_gbrow = cT(P_const, [1, H], "gbrow")
            nc.sync.dma_start(sb_gbrow, gbrow[:])
            cst_005 = cT(P_const, [128, 1], "cst005")
            V.memset(cst_005, 0.05)
            cst_eps = cT(P_const, [128, 1], "csteps")
            V.memset(cst_eps, EPS)

            sb_wot = cT(P_keep, [128, 2, H], "wot")
            for dt in range(2):
                nc.sync.dma_start(sb_wot[:, dt, :],
                                  wot[128 * dt:128 * (dt + 1), :])
            sb_gwt = cT(P_keep, [128, 8, H], "gwt")
            for kb in range(8):
                nc.sync.dma_start(sb_gwt[:, kb, :],
                                  gwt[128 * kb:128 * (kb + 1), :])

            st = {}
            for nm, src in (("w1t", w1t0), ("w2", w2_0), ("w2t", w2t0)):
                t_ = cT(P_keep, [128, 2, D], f"st_{nm}")
                for dt in range(2):
                    nc.sync.dma_start(t_[:, dt, :],
                                      src[128 * dt:128 * (dt + 1), :])
                st[nm] = t_
            for nm in ("s1t", "s2", "s2t"):
                t_ = cT(P_keep, [128, 2, D], f"st_{nm}")
                V.memset(t_, 0.0)
                st[nm] = t_

            ng_bc = cT(P_keep, [128, H], "ngbc")

            # long-lived phase-1 products
            hidT = cT(P_keep, [128, 8, 128], "hidT")
            gates_own = cT(P_keep, [128, 3], "gates_own")
            kvq_own = cT(P_keep, [128, 3, 2, 128], "kvq_own")

            # ======================= PHASE 1 =======================
            with tc.tile_pool(name="ph1", bufs=1) as P1, \
                 tc.tile_pool(name="ph1w", bufs=2) as P1w:
                sb_bcat = cT(P1, [1, 1536], "bcat")
                nc.sync.dma_start(sb_bcat, bcat[:])
                sb_ngrow = cT(P1, [1, H], "ngrow")
                nc.sync.dma_start(sb_ngrow, ngrow[:])
                for half in range(2):
                    ps = psA(128, "ngbc_ps")
                    MM(ps, ones2[0:1, :],
                       sb_ngrow[:, half * 512:(half + 1) * 512],
                       start=True, stop=True)
                    nc.scalar.copy(ng_bc[:, half * 512:(half + 1) * 512], ps)
                sb_hown = cT(P1, [128, H], "hown")
                nc.sync.dma_start(sb_hown, hidown[:])
                sb_hhalo = cT(P1, [3, H], "hhalo")
                nc.sync.dma_start(sb_hhalo, hidhalo[:])

                for kb in range(8):
                    ps = psA(128, "tp")
                    nc.tensor.transpose(ps[:, 0:128],
                                        sb_hown[:, kb * 128:(kb + 1) * 128],
                                        sb_ident)
                    nc.scalar.copy(hidT[:, kb, :], ps[:, 0:128])
                hidTh = cT(P1, [128, 8, 3], "hidTh")
                for kb in range(8):
                    ps = psA(128, "tph")
                    nc.tensor.transpose(ps[:, 0:3],
                                        sb_hhalo[0:3, kb * 128:(kb + 1) * 128],
                                        sb_ident[0:3, 0:3])
                    nc.scalar.copy(hidTh[:, kb, :], ps[:, 0:3])

                # own projections (token-major), split into 3 psum tiles;
                # weight k-tiles stream through a rotating buffer
                psP = [psA(128, f"psP{ns}") for ns in range(3)]
                psH = [psA(3, "psH0"), psA(3, "psH1")]
                for kb in range(8):
                    wt = P1w.tile([128, 1536], f32, tag="wk", name="wk", bufs=3)
                    nc.sync.dma_start(wt, wcat[128 * kb:128 * (kb + 1), :])
                    for ns in range(3):
                        MM(psP[ns], hidT[:, kb, :],
                           wt[:, ns * 512:(ns + 1) * 512],
                           start=(kb == 0), stop=False)
                    MM(psH[0], hidTh[:, kb, :], wt[:, 0:512],
                       start=(kb == 0), stop=(kb == 7))
                    MM(psH[1][:, 0:256], hidTh[:, kb, :], wt[:, 512:768],
                       start=(kb == 0), stop=(kb == 7))
                for ns in range(3):
                    MM(psP[ns], ones2[0:1, :],
                       sb_bcat[:, ns * 512:(ns + 1) * 512],
                       start=False, stop=True)

                # gates from own proj
                scr1 = cT(P1w, [128, 256], "scr1")
                acc_a = cT(P_cols, [128, 1], "acc_a")
                ACT(scr1, psP[1][:, 256:512], AF.Sigmoid, accum_out=acc_a)
                scr2 = cT(P1w, [128, 256], "scr2")
                acc_e = cT(P_cols, [128, 1], "acc_e")
                ACT(scr2, psP[2][:, 0:256], AF.Sigmoid, accum_out=acc_e)
                scr3 = cT(P1w, [128, 256], "scr3")
                acc_t = cT(P_cols, [128, 1], "acc_t")
                # softplus(x) = -ln(sigmoid(-x)); accumulate the ln
                ACT(scr3, psP[2][:, 256:512], AF.Sigmoid, scale=-1.0)
                scr4 = cT(P1w, [128, 256], "scr4")
                ACT(scr4, scr3, AF.Ln, accum_out=acc_t)
                ACT(gates_own[:, 0:1], acc_e, AF.Ln, bias=cst_005[:, 0:1], scale=0.9 / 256.0)
                ACT(gates_own[:, 1:2], acc_a, AF.Ln, bias=1.0, scale=-0.1 / 256.0)
                nc.scalar.mul(gates_own[:, 2:3], acc_t, -LR / 256.0)

                pP1 = cT(P1, [128, 768], "pP1")
                V.tensor_copy(pP1[:, 0:512], psP[0])
                V.tensor_copy(pP1[:, 512:768], psP[1][:, 0:256])
                pH1 = cT(P1, [3, 768], "pH1")
                V.tensor_copy(pH1[:, 0:512], psH[0])
                V.tensor_copy(pH1[:, 512:768], psH[1][:, 0:256])

                xfull = cT(P1, [128, 3, 2, 131], "xfull")
                for pj in range(3):
                    for dt in range(2):
                        c0 = pj * 256 + dt * 128
                        ps = psA(128, "tp")
                        nc.tensor.transpose(ps[:, 0:128], pP1[:, c0:c0 + 128],
                                            sb_ident)
                        nc.scalar.copy(xfull[:, pj, dt, 3:131], ps[:, 0:128])
                        ps2 = psA(128, "tph")
                        nc.tensor.transpose(ps2[:, 0:3], pH1[0:3, c0:c0 + 128],
                                            sb_ident[0:3, 0:3])
                        nc.scalar.copy(xfull[:, pj, dt, 0:3], ps2[:, 0:3])

                for pj in range(3):
                    for dt in range(2):
                        x = xfull[:, pj, dt, :]
                        wb = dt * 12 + pj * 4
                        a0 = cT(P1w, [128, 128], "cv0")
                        V.tensor_scalar_mul(a0, x[:, 0:128],
                                            sb_convw[:, wb:wb + 1])
                        a1 = cT(P1w, [128, 128], "cv1")
                        V.scalar_tensor_tensor(a1, x[:, 1:129],
                                               sb_convw[:, wb + 1:wb + 2],
                                               a0, OP.mult, OP.add)
                        a2 = cT(P1w, [128, 128], "cv2")
                        V.scalar_tensor_tensor(a2, x[:, 2:130],
                                               sb_convw[:, wb + 2:wb + 3],
                                               a1, OP.mult, OP.add)
                        a3 = cT(P1w, [128, 128], "cv3")
                        V.scalar_tensor_tensor(a3, x[:, 3:131],
                                               sb_convw[:, wb + 3:wb + 4],
                                               a2, OP.mult, OP.add)
                        V.tensor_scalar_add(kvq_own[:, pj, dt, :], a3,
                                            sb_convb[:, dt * 3 + pj:
                                                     dt * 3 + pj + 1])

                if debug:
                    for pj in range(3):
                        nc.sync.dma_start(dbg_kvq[:, pj], kvq_own[:, pj])
                    nc.sync.dma_start(dbg_gates[:], gates_own[:])

                nc.sync.dma_start(
                    agin[:, 0:512],
                    kvq_own[:, 0:2, :, :].rearrange("p a b t -> p (a b t)"))
                nc.sync.dma_start(agin[:, 512:515], gates_own[:])

            # ======================= ALLGATHER =======================
            nc.gpsimd.collective_compute(
                "AllGather", mybir.AluOpType.bypass,
                ins=[agin[:]], outs=[agout[:]],
                replica_groups=[list(range(N_CORES))],
            )

            K_fm = [cT(P_keep, [128, 2, T], f"kfm{cc}") for cc in range(2)]
            V_fm = [cT(P_keep, [128, 2, T], "vfm") for cc in range(2)]
            for cc in range(2):
                for rr in range(4):
                    rk = 4 * cc + rr
                    nc.sync.dma_start(
                        K_fm[cc][:, :, rr * 128:(rr + 1) * 128],
                        agout[128 * rk:128 * (rk + 1), 0:256]
                        .rearrange("p (dt t) -> p dt t", dt=2))
                    nc.sync.dma_start(
                        V_fm[cc][:, :, rr * 128:(rr + 1) * 128],
                        agout[128 * rk:128 * (rk + 1), 256:512]
                        .rearrange("p (dt t) -> p dt t", dt=2))
            gates_all = cT(P_keep, [128, 8, 3], "gates_all")
            for rk in range(N_CORES):
                nc.sync.dma_start(gates_all[:, rk, :],
                                  agout[128 * rk:128 * (rk + 1), 512:515])

            # helper: (1,128) row slice -> (128,1) col
            def row_to_col(row_ap, out_col):
                ps = psA(128, "r2c")
                MM(ps[:, 0:1], row_ap, sb_ident[0:1, 0:1], start=True, stop=True)
                nc.scalar.copy(out_col, ps[:, 0:1])

            # ---------- local row machinery (chunk-independent) ----------
            bs_ps = psA(1, "bs")
            MM(bs_ps[:, 0:8], ones2[:, 0:1], gates_all[:, :, 1:2],
               start=True, stop=True)
            pre_mul = cT(P_cols, [1, 8], "pre_mul")
            V.tensor_tensor(pre_mul, bs_ps[:, 0:8], sb_maskvec, OP.mult)
            prefix = cT(P_cols, [1, 1], "prefix")
            V.tensor_reduce(prefix, pre_mul, X, OP.add)

            l1m_row = cT(P_cols, [1, 128], "l1m_row")
            psx = psA(1, "l1mr")
            MM(psx[:, 0:128], gates_own[:, 1:2], sb_ident, start=True, stop=True)
            nc.scalar.copy(l1m_row, psx[:, 0:128])
            la_my_row = cT(P_cols, [1, 128], "la_my_row")
            V.tensor_tensor_scan(la_my_row, ones_row512[:, 0:128], l1m_row,
                                 prefix, OP.mult, OP.add)
            la_my_col = cT(P_cols, [128, 1], "la_my_col")
            row_to_col(la_my_row, la_my_col)
            a_col_my = cT(P_cols, [128, 1], "a_col_my")
            ACT(a_col_my, la_my_col, AF.Exp)
            la_my_bc = cT(P_keep, [128, 128], "la_my_bc")
            psx = psA(128, "lamybc")
            MM(psx[:, 0:128], ones2[0:1, :], la_my_row, start=True, stop=True)
            nc.scalar.copy(la_my_bc, psx[:, 0:128])
            mypos_bc = cT(P_keep, [128, 128], "mypos_bc")
            psx = psA(128, "myposbc")
            MM(psx[:, 0:128], ones2[0:1, :], sb_mypos, start=True, stop=True)
            nc.scalar.copy(mypos_bc, psx[:, 0:128])

            with tc.tile_pool(name="workp", bufs=1) as P_work:
                ret = cT(P_keep, [128, 2, 128], "ret")
                for cc in range(2):
                    Kc, Vc = K_fm[cc], V_fm[cc]
                    th_cols = gates_all[:, 4 * cc:4 * cc + 4, 2:3].rearrange("p a o -> p (a o)")

                    # ---- chunk scalar rows/cols ----
                    g_rows = []
                    for gi in range(2):
                        ps = psA(4, f"gr{gi}")
                        MM(ps[:, 0:128],
                           gates_all[:, 4 * cc:4 * cc + 4, gi:gi + 1],
                           sb_ident, start=True, stop=True)
                        gsb = cT(P_cols, [4, 128], f"gsb{gi}")
                        nc.scalar.copy(gsb, ps[:, 0:128])
                        row = cT(P_cols, [1, T], f"graw{gi}")
                        for rr in range(4):
                            nc.sync.dma_start(row[:, rr * 128:(rr + 1) * 128],
                                              gsb[rr:rr + 1, :])
                        srow = cT(P_cols, [1, T], f"gcum{gi}")
                        V.tensor_tensor_scan(srow, ones_row512, row, 0.0,
                                             OP.mult, OP.add)
                        g_rows.append(srow)
                    le_row, la_row = g_rows

                    le_cols = cT(P_cols, [128, 4], "le_cols")
                    la_cols = cT(P_cols, [128, 4], "la_cols")
                    for b in range(4):
                        row_to_col(le_row[:, b * 128:(b + 1) * 128],
                                   le_cols[:, b:b + 1])
                        row_to_col(la_row[:, b * 128:(b + 1) * 128],
                                   la_cols[:, b:b + 1])
                    E_cols = cT(P_cols, [128, 4], "E_cols")
                    ACT(E_cols, le_cols, AF.Exp)

                    le_bc = psA(128, "lebc")
                    MM(le_bc, ones2[0:1, :], le_row, start=True, stop=True)
                    G = cT(P_work, [128, 4, T], "G")
                    for b in range(4):
                        scrg = psA(128, "scrg")
                        V.tensor_scalar(scrg, le_bc, le_cols[:, b:b + 1], 0.0,
                                        OP.subtract, OP.max)
                        ACT(G[:, b, :], scrg, AF.Exp, scale=-1.0)
                        V.tensor_tensor(G[:, b, b * 128:(b + 1) * 128],
                                        G[:, b, b * 128:(b + 1) * 128],
                                        sb_tril, OP.mult)

                    # ---- grads at chunk-start state ----
                    w1t_c, w2_c, w2t_c = st["w1t"], st["w2"], st["w2t"]
                    s1t_c, s2t_c = st["s1t"], st["s2t"]

                    Z_fm = psB([128, 2, T], "Z_fm")
                    for m in range(2):
                        for kt in range(2):
                            MM(Z_fm[:, m, :],
                               w1t_c[:, kt, m * 128:(m + 1) * 128],
                               Kc[:, kt, :], start=(kt == 0), stop=(kt == 1))
                    Hp_fm = cT(P_work, [128, 2, T], "hpfm")
                    ACT(Hp_fm[:, 0, :], Z_fm[:, 0, :], AF.Silu)
                    ACT(Hp_fm[:, 1, :], Z_fm[:, 1, :], AF.Silu)

                    Z_tm = [psA(128, "ztm0"), psA(128, "ztm1")]
                    for b in range(4):
                        for kt in range(2):
                            MM(Z_tm[b // 2][:, (b % 2) * 256:(b % 2 + 1) * 256],
                               Kc[:, kt, b * 128:(b + 1) * 128],
                               w1t_c[:, kt, :], start=(kt == 0), stop=(kt == 1))
                    Hp_tm = cT(P_work, [128, 4, D], "hptm")
                    sp_tm = cT(P_work, [128, 4, D], "sptm")
                    for b in range(4):
                        zt = Z_tm[b // 2][:, (b % 2) * 256:(b % 2 + 1) * 256]
                        ACT(Hp_tm[:, b, :], zt, AF.Silu)
                        ACT(sp_tm[:, b, :], zt, AF.Derivative_silu)

                    Pred = psB([128, 2, T], "Pred")
                    for m in range(2):
                        for kt in range(2):
                            MM(Pred[:, m, :],
                               w2t_c[:, kt, m * 128:(m + 1) * 128],
                               Hp_fm[:, kt, :], start=(kt == 0), stop=(kt == 1))
                    PV_fm = cT(P_work, [128, 2, T], "pvfm")
                    V.tensor_sub(PV_fm[:, 0, :], Pred[:, 0, :], Vc[:, 0, :])
                    V.tensor_sub(PV_fm[:, 1, :], Pred[:, 1, :], Vc[:, 1, :])

                    dH_tm = [psA(128, "dhtm0"), psA(128, "dhtm1")]
                    for b in range(4):
                        for kt in range(2):
                            MM(dH_tm[b // 2][:, (b % 2) * 256:(b % 2 + 1) * 256],
                               PV_fm[:, kt, b * 128:(b + 1) * 128],
                               w2_c[:, kt, :], start=(kt == 0), stop=(kt == 1))
                    dzpp = cT(P_work, [128, 4, D], "dzpp")
                    nzh_cols = cT(P_cols, [128, 4], "nzh_cols")
                    for b in range(4):
                        V.tensor_tensor(
                            dzpp[:, b, :],
                            dH_tm[b // 2][:, (b % 2) * 256:(b % 2 + 1) * 256],
                            sp_tm[:, b, :], OP.mult)
                        scrD = psA(128, "scrD")
                        ACT(scrD[:, 0:256], dzpp[:, b, :], AF.Square,
                            accum_out=nzh_cols[:, b:b + 1])

                    ncols = {}
                    for nm, src in (("nk", Kc), ("npv", PV_fm), ("nh", Hp_fm)):
                        psn = psA(1, f"nrow_{nm}")
                        for kt in range(2):
                            sq = cT(P_work, [128, T], "sq")
                            ACT(sq, src[:, kt, :], AF.Square)
                            MM(psn[:, 0:512], ones2[:, 0:1], sq,
                               start=(kt == 0), stop=(kt == 1))
                        nr = cT(P_cols, [1, T], f"nr_{nm}")
                        nc.scalar.copy(nr, psn[:, 0:512])
                        cl = cT(P_cols, [128, 4], f"nc_{nm}")
                        for b in range(4):
                            row_to_col(nr[:, b * 128:(b + 1) * 128],
                                       cl[:, b:b + 1])
                        ncols[nm] = cl

                    cs_cols = cT(P_cols, [128, 4], "cs_cols")
                    g1 = cT(P_cols, [128, 4], "g1c")
                    V.tensor_tensor(g1, nzh_cols, ncols["nk"], OP.mult)
                    g2 = cT(P_cols, [128, 4], "g2c")
                    V.tensor_tensor(g2, ncols["npv"], ncols["nh"], OP.mult)
                    V.tensor_add(g1, g1, g2)
                    ACT(g1, g1, AF.Sqrt, scale=4.0 * GS * GS)
                    V.tensor_scalar_max(g1, g1, EPS)
                    V.reciprocal(g1, g1)
                    V.tensor_scalar_min(g1, g1, 1.0)
                    V.tensor_tensor(g1, g1, th_cols, OP.mult)
                    V.tensor_scalar_mul(cs_cols, g1, 2.0 * GS)
                    if debug:
                        nc.sync.dma_start(dbg_cs[:, :, cc], cs_cols[:])

                    for b in range(4):
                        nc.scalar.mul(dzpp[:, b, :], dzpp[:, b, :],
                                      cs_cols[:, b:b + 1])
                    rpp = cT(P_work, [128, 4, D], "rpp")
                    for b in range(4):
                        for dt in range(2):
                            ps = psA(128, "tp")
                            nc.tensor.transpose(
                                ps[:, 0:128],
                                PV_fm[:, dt, b * 128:(b + 1) * 128], sb_ident)
                            nc.scalar.mul(rpp[:, b, dt * 128:(dt + 1) * 128],
                                          ps[:, 0:128], cs_cols[:, b:b + 1])
                    if debug:
                        for b in range(4):
                            nc.sync.dma_start(dbg_dzpp[:, b, :, cc],
                                              dzpp[:, b, :])
                            nc.sync.dma_start(dbg_rpp[:, b, :, cc],
                                              rpp[:, b, :])

                    # ---- state update (chunk 0 only) ----
                    if cc == 0:
                        leT = le_row[:, T - 1:T]
                        laT = la_row[:, T - 1:T]
                        gse_row = cT(P_rows, [1, T], "gse_row")
                        V.tensor_scalar_sub(gse_row, le_row, leT)
                        ACT(gse_row, gse_row, AF.Exp, scale=-1.0)
                        cwL_row = cT(P_rows, [1, T], "cwL_row")
                        V.tensor_scalar_sub(cwL_row, la_row, laT)
                        ACT(cwL_row, cwL_row, AF.Exp, scale=-1.0)
                        gse_cols = cT(P_cols, [128, 4], "gse_cols")
                        cwL_cols = cT(P_cols, [128, 4], "cwL_cols")
                        for b in range(4):
                            row_to_col(gse_row[:, b * 128:(b + 1) * 128],
                                       gse_cols[:, b:b + 1])
                            row_to_col(cwL_row[:, b * 128:(b + 1) * 128],
                                       cwL_cols[:, b:b + 1])

                        cw_cols = cT(P_cols, [128, 4], "cw_cols")
                        for a in range(4):
                            ps = psA(128, "cw")
                            for b in range(a, 4):
                                MM(ps[:, 0:1], G[:, b, a * 128:(a + 1) * 128],
                                   cwL_cols[:, b:b + 1],
                                   start=(b == a), stop=(b == 3))
                            nc.scalar.copy(cw_cols[:, a:a + 1], ps[:, 0:1])

                        tmp14 = cT(P_cols, [128, 4], "tmp14")
                        V.tensor_tensor(tmp14, cwL_cols, E_cols, OP.mult)
                        psx = psA(1, "bt")
                        MM(psx[:, 0:4], ones2[:, 0:1], tmp14,
                           start=True, stop=True)
                        sc3 = cT(P_cols, [1, 3], "sc3")
                        V.tensor_reduce(sc3[:, 2:3], psx[:, 0:4], X, OP.add)
                        ACT(sc3[:, 0:1], leT, AF.Exp)
                        ACT(sc3[:, 1:2], laT, AF.Exp)
                        psx = psA(128, "sccols")
                        MM(psx[:, 0:3], ones2[0:1, :], sc3, start=True, stop=True)
                        sc_cols = cT(P_cols, [128, 3], "sc_cols")
                        nc.scalar.copy(sc_cols, psx[:, 0:3])
                        ET_col = sc_cols[:, 0:1]
                        AT_col = sc_cols[:, 1:2]
                        bT_col = sc_cols[:, 2:3]

                        K_S = cT(P_work, [128, 4, D], "K_S")
                        K_C = cT(P_work, [128, 4, D], "K_C")
                        for b in range(4):
                            for dt in range(2):
                                ps = psA(128, "tp")
                                nc.tensor.transpose(
                                    ps[:, 0:128],
                                    Kc[:, dt, b * 128:(b + 1) * 128], sb_ident)
                                nc.scalar.mul(
                                    K_S[:, b, dt * 128:(dt + 1) * 128],
                                    ps[:, 0:128], gse_cols[:, b:b + 1])
                                nc.scalar.mul(
                                    K_C[:, b, dt * 128:(dt + 1) * 128],
                                    ps[:, 0:128], cw_cols[:, b:b + 1])
                        Hp_S = cT(P_work, [128, 4, D], "Hp_S")
                        Hp_C = cT(P_work, [128, 4, D], "Hp_C")
                        for b in range(4):
                            nc.scalar.mul(Hp_S[:, b, :], Hp_tm[:, b, :],
                                          gse_cols[:, b:b + 1])
                            nc.scalar.mul(Hp_C[:, b, :], Hp_tm[:, b, :],
                                          cw_cols[:, b:b + 1])

                        new_st = {}
                        for nm, lh, rh, snm in (
                            ("s1t", K_S, dzpp, None), ("w1t", K_C, dzpp, "s1t"),
                            ("s2", rpp, Hp_S, None), ("w2", rpp, Hp_C, "s2"),
                            ("s2t", Hp_S, rpp, None), ("w2t", Hp_C, rpp, "s2t"),
                        ):
                            pu = psA(128, f"upd_{nm}")
                            # accumulate both halves (m=0,1) into one 2KB slot
                            for m in range(2):
                                for b in range(4):
                                    MM(pu[:, m * 256:(m + 1) * 256],
                                       lh[:, b, m * 128:(m + 1) * 128],
                                       rh[:, b, :],
                                       start=(b == 0), stop=(b == 3))
                            t_ = cT(P_keep, [128, 2, D], f"nst_{nm}")
                            tflat = t_.rearrange("p a o -> p (a o)")
                            oflat = st[nm].rearrange("p a o -> p (a o)")
                            tmp = cT(P_work, [128, 2 * D], "sttmp")
                            if snm is None:
                                nc.scalar.mul(tmp, oflat, ET_col)
                            else:
                                nc.scalar.mul(tmp, oflat, AT_col)
                                V.scalar_tensor_tensor(
                                    tmp, st[snm].rearrange("p a o -> p (a o)"),
                                    bT_col, tmp, OP.mult, OP.add)
                            V.tensor_sub(tflat, tmp, pu[:, 0:512])
                            new_st[nm] = t_
                        if debug:
                            for si, nm in enumerate(("w1t", "s1t", "w2", "w2t",
                                                     "s2", "s2t")):
                                for dt in range(2):
                                    nc.sync.dma_start(dbg_st[:, dt, :, si],
                                                      new_st[nm][:, dt, :])

                    # ---- retrieval for my 128 rows ----
                    L_my = cT(P_work, [128, 4, 128], "lmy")
                    for b in range(4):
                        scrm = cT(P_work, [128, 128], "scrm")
                        V.tensor_scalar(scrm, la_my_bc, la_cols[:, b:b + 1],
                                        0.0, OP.subtract, OP.min)
                        ACT(scrm, scrm, AF.Exp)
                        scrm2 = cT(P_work, [128, 128], "scrm2")
                        V.tensor_scalar(scrm2, mypos_bc, sb_poscol[:, b:b + 1],
                                        0.0, OP.subtract, OP.is_ge)
                        V.tensor_tensor(L_my[:, b, :], scrm, scrm2, OP.mult)

                    psb_ = psA(1, "brow")
                    for b in range(4):
                        tmpL = cT(P_work, [128, 128], "tmpL")
                        V.tensor_scalar_mul(tmpL, L_my[:, b, :],
                                            E_cols[:, b:b + 1])
                        MM(psb_[:, 0:128], ones2[:, 0:1], tmpL,
                           start=(b == 0), stop=(b == 3))
                    b_row = cT(P_cols, [1, 128], "b_row")
                    nc.scalar.copy(b_row, psb_[:, 0:128])
                    b_col_my = cT(P_cols, [128, 1], "b_col_my")
                    row_to_col(b_row, b_col_my)

                    q_own = kvq_own[:, 2, :, :]

                    QK = psA(128, "qk")
                    for a in range(4):
                        for dt in range(2):
                            MM(QK[:, a * 128:(a + 1) * 128],
                               Kc[:, dt, a * 128:(a + 1) * 128],
                               q_own[:, dt, :], start=(dt == 0), stop=(dt == 1))
                    CTp = psA(128, "ct")
                    for a in range(4):
                        for b in range(a, 4):
                            MM(CTp[:, a * 128:(a + 1) * 128],
                               G[:, b, a * 128:(a + 1) * 128],
                               L_my[:, b, :], start=(b == a), stop=(b == 3))
                    CT_sb = cT(P_work, [128, 4, 128], "ctsb")
                    nc.scalar.copy(CT_sb.rearrange("p a t -> p (a t)"), CTp)
                    CQK = cT(P_work, [128, 4, 128], "cqk")
                    V.tensor_tensor(CQK.rearrange("p a t -> p (a t)"),
                                    CT_sb.rearrange("p a t -> p (a t)"),
                                    QK, OP.mult)

                    U1 = psA(128, "u1")
                    for a in range(4):
                        MM(U1[:, 0:256], CQK[:, a, :], dzpp[:, a, :],
                           start=(a == 0), stop=(a == 3))
                    X1 = psA(128, "x1")
                    for dt in range(2):
                        MM(X1[:, 0:256], q_own[:, dt, :], w1t_c[:, dt, :],
                           start=(dt == 0), stop=(dt == 1))
                    X2 = psA(128, "x2")
                    for dt in range(2):
                        MM(X2[:, 0:256], q_own[:, dt, :], s1t_c[:, dt, :],
                           start=(dt == 0), stop=(dt == 1))
                    u1sb = cT(P_work, [128, D], "u1sb")
                    nc.scalar.copy(u1sb, U1[:, 0:256])
                    y1 = cT(P_work, [128, D], "y1")
                    V.scalar_tensor_tensor(y1, X2[:, 0:256], b_col_my, u1sb,
                                           OP.mult, OP.subtract)
                    V.scalar_tensor_tensor(y1, X1[:, 0:256], a_col_my, y1,
                                           OP.mult, OP.add)
                    H_tm = cT(P_work, [128, D], "htm")
                    ACT(H_tm, y1, AF.Silu)
                    H_fm = cT(P_work, [128, 2, 128], "hfm")
                    for dt in range(2):
                        ps = psA(128, "tp")
                        nc.tensor.transpose(ps[:, 0:128],
                                            H_tm[:, dt * 128:(dt + 1) * 128],
                                            sb_ident)
                        nc.scalar.copy(H_fm[:, dt, :], ps[:, 0:128])

                    HHp = psA(128, "qk2")
                    for a in range(4):
                        for dt in range(2):
                            MM(HHp[:, a * 128:(a + 1) * 128],
                               Hp_fm[:, dt, a * 128:(a + 1) * 128],
                               H_fm[:, dt, :], start=(dt == 0), stop=(dt == 1))
                    CHHp = cT(P_work, [128, 4, 128], "chhp")
                    V.tensor_tensor(CHHp.rearrange("p a t -> p (a t)"),
                                    CT_sb.rearrange("p a t -> p (a t)"),
                                    HHp, OP.mult)

                    U2 = psA(128, "u2")
                    for a in range(4):
                        MM(U2[:, 0:256], CHHp[:, a, :], rpp[:, a, :],
                           start=(a == 0), stop=(a == 3))
                    X3 = psA(128, "x3")
                    for dt in range(2):
                        MM(X3[:, 0:256], H_fm[:, dt, :], w2t_c[:, dt, :],
                           start=(dt == 0), stop=(dt == 1))
                    X4 = psA(128, "x4")
                    for dt in range(2):
                        MM(X4[:, 0:256], H_fm[:, dt, :], s2t_c[:, dt, :],
                           start=(dt == 0), stop=(dt == 1))
                    u2sb = cT(P_work, [128, D], "u2sb")
                    nc.scalar.copy(u2sb, U2[:, 0:256])
                    y2 = cT(P_work, [128, D], "y2")
                    V.scalar_tensor_tensor(y2, X4[:, 0:256], b_col_my, u2sb,
                                           OP.mult, OP.subtract)
                    V.scalar_tensor_tensor(y2, X3[:, 0:256], a_col_my, y2,
                                           OP.mult, OP.add)
                    if debug:
                        nc.sync.dma_start(dbg_y2[:, :, cc], y2[:])

                    for dt in range(2):
                        ps = psA(128, "tp")
                        nc.tensor.transpose(ps[:, 0:128],
                                            y2[:, dt * 128:(dt + 1) * 128],
                                            sb_ident)
                        if cc == 0:
                            nc.scalar.mul(ret[:, dt, :], ps[:, 0:128],
                                          sb_mysel[:, 0:1])
                        else:
                            V.scalar_tensor_tensor(ret[:, dt, :], ps[:, 0:128],
                                                   sb_mysel[:, 1:2],
                                                   ret[:, dt, :],
                                                   OP.mult, OP.add)

                    if cc == 0:
                        st = new_st

                if debug:
                    nc.sync.dma_start(dbg_ret[:], ret[:])

                # ======================= PHASE 3 =======================
                po = psB([128, H], "po")
                for half in range(2):
                    for dt in range(2):
                        MM(po[:, half * 512:(half + 1) * 512], ret[:, dt, :],
                           sb_wot[:, dt, half * 512:(half + 1) * 512],
                           start=(dt == 0), stop=(dt == 1))
                ssq = cT(P_cols, [128, 1], "ssq")
                ss2 = cT(P_cols, [128, 1], "ss2")
                sqh = cT(P_work, [128, T], "sq")
                ACT(sqh, po[:, 0:512], AF.Square, accum_out=ssq)
                sqh2 = cT(P_work, [128, T], "sq")
                ACT(sqh2, po[:, 512:1024], AF.Square, accum_out=ss2)
                V.tensor_add(ssq, ssq, ss2)
                rstd = cT(P_cols, [128, 1], "rstd")
                ACT(rstd, ssq, AF.Sqrt, bias=cst_eps[:, 0:1], scale=1.0 / H)
                V.reciprocal(rstd, rstd)
                normed = cT(P_work, [128, H], "sq")
                nc.scalar.mul(normed, po, rstd)
                V.tensor_tensor(normed, normed, ng_bc, OP.mult)

                gate = psB([128, H], "gate")
                for half in range(2):
                    for kb in range(8):
                        MM(gate[:, half * 512:(half + 1) * 512], hidT[:, kb, :],
                           sb_gwt[:, kb, half * 512:(half + 1) * 512],
                           start=(kb == 0), stop=False)
                    MM(gate[:, half * 512:(half + 1) * 512], ones2[0:1, :],
                       sb_gbrow[:, half * 512:(half + 1) * 512],
                       start=False, stop=True)
                V.tensor_tensor(normed, normed, gate, OP.mult)
                nc.sync.dma_start(outp[:], normed)

    _split_excess_waits(nc, lim=1)
    return nc


# ---------------------------------------------------------------- host side
def _prep_inputs(inputs):
    ii = {k: np.ascontiguousarray(np.asarray(v), dtype=np.float32)
          for k, v in inputs.items()}
    hidden = ii["hidden"].reshape(S, H)

    wcat = np.concatenate([ii["Wk"].T, ii["Wv"].T, ii["Wq"].T,
                           ii["aW"].T, ii["eW"].T, ii["tW"].T], axis=1)
    bcat = np.concatenate([np.zeros(768, np.float32),
                           ii["ab"], ii["eb"], ii["tb"]])[None]
    convw = np.zeros((128, 24), np.float32)
    convb = np.zeros((128, 6), np.float32)
    for dt in range(2):
        for pj, (w, b) in enumerate(((ii["ckw"], ii["ckb"]),
                                     (ii["cvw"], ii["cvb"]),
                                     (ii["cqw"], ii["cqb"]))):
            convw[:, dt * 12 + pj * 4:dt * 12 + pj * 4 + 4] = \
                w[dt * 128:(dt + 1) * 128]
            convb[:, dt * 3 + pj] = b[dt * 128:(dt + 1) * 128]

    shared = {
        "wcat": np.ascontiguousarray(wcat),
        "bcat": bcat, "convw": convw, "convb": convb,
        "w1t0": np.ascontiguousarray(ii["mW1"].T),
        "w2_0": ii["mW2"],
        "w2t0": np.ascontiguousarray(ii["mW2"].T),
        "wot": np.ascontiguousarray(ii["Wo"].T),
        "gwt": np.ascontiguousarray(ii["gW"].T),
        "ngrow": ii["norm_g"][None],
        "gbrow": ii["gb"][None],
        "tril": np.tril(np.ones((128, 128), np.float32)),
        "ident": np.eye(128, dtype=np.float32),
        "poscol": np.ascontiguousarray(
            (np.arange(128, dtype=np.float32)[:, None]
             + 128.0 * np.arange(4, dtype=np.float32)[None, :])),
    }
    in_maps = []
    for c in range(N_CORES):
        g, r = c // 4, c % 4
        start = 128 * c
        halo = np.zeros((3, H), np.float32)
        if start >= 3:
            halo[:] = hidden[start - 3:start]
        m = dict(shared)
        m["hidown"] = np.ascontiguousarray(hidden[start:start + 128])
        m["hidhalo"] = halo
        m["mypos"] = np.ascontiguousarray(
            (128.0 * r + np.arange(128, dtype=np.float32))[None])
        mv = np.zeros(8, np.float32)
        for j in range(N_CORES):
            if j // 4 == g and j % 4 < r:
                mv[j] = 1.0
        m["maskvec"] = mv[None]
        sel = np.zeros((128, 2), np.float32)
        sel[:, g] = 1.0
        m["mysel"] = np.ascontiguousarray(sel)
        in_maps.append(m)
    return in_maps


def kernel(**inputs):
    from concourse.bass_utils import run_bass_kernel_spmd

    debug = bool(inputs.pop("_debug", False))
    key = ("prog", debug)
    if key not in _CACHE:
        _CACHE[key] = _build_program(debug=debug)
    nc = _CACHE[key]

    in_maps = _prep_inputs(inputs)
    res = run_bass_kernel_spmd(nc, in_maps, core_ids=list(range(N_CORES)))
    out = np.concatenate([res.results[c]["outp"] for c in range(N_CORES)],
                         axis=0)[None]
    if debug:
        return out.astype(np.float32), res
    return out.astype(np.float32)


# revision 20
# speedup vs baseline: 291.6291x; 291.6291x over previous
"""Trainium2 Bass kernel for nn_NeuralLongTermMemory (chunked fast-weight scan).

The per-token fast-weight update is a linear recurrence with per-token scalar
coefficients and rank-1 gradient increments, so each 512-token chunk collapses
into dense matmuls (chunked linear-attention form).  8 cores run one uniform
SPMD program: phase 1 (projections+gates) and phase 3 (output proj + RMSNorm +
gate) are token-parallel (128 tokens/core); the chunk-level column side (grads
at chunk-start weights, decay matrices, state update) is replicated on every
core from an AllGather of k/v/gates; each core computes retrieval rows only
for its own 128 tokens (both chunk passes; the wrong-chunk pass is discarded
by a data-driven select mask).
"""
import numpy as np

N_CORES = 8
H = 1024
D = 256
T = 512
S = 1024
LR = 0.1
GS = 0.1
EPS = 1e-6

_CACHE = {}
REGIONS = []


# ---------------------------------------------------------------- tile patch
def _patch_tile_drain():
    """This walrus build rejects >1 semaphore wait per instruction; split the
    TileContext exit drain's waits across single-wait NOPs."""
    import concourse.mybir as mybir
    import concourse.tile as tile_mod
    from concourse.tile import TileContext

    if getattr(TileContext, "_nltm_patched", False):
        return

    def _drain_and_barrier(self, tick_clock, wait_clock):
        nc = self.nc
        probe = nc.sync.nop(hint="drain_wait_probe", nofuse=True)
        if probe.ins.sync_info is None:
            probe.ins.sync_info = mybir.SyncInfo(on_wait=[], on_update=[])
        wait_clock.add_sem_waits(
            probe.ins, tile_mod.ScopedClock({None: tick_clock.global_clock}))
        waits = list(probe.ins.sync_info.on_wait or [])
        probe.ins.sync_info.on_wait.clear()
        for w in waits:
            nop = nc.sync.nop(hint="drain_wait_split", nofuse=True)
            nop.ins.sync_info = mybir.SyncInfo(on_wait=[w], on_update=[])
        nc.sync.drain()
        nc.all_engine_barrier()
        assert self.sems is not None
        popped = nc._tile_sem_poison_stack.pop()
        assert popped is self._sem_poison
        nc.clear_and_free_semaphores(list(self.sems.allocated().values()))
        nc.all_engine_barrier()

    TileContext._drain_and_barrier = _drain_and_barrier
    TileContext._nltm_patched = True


def _split_excess_waits(nc, lim=1):
    import concourse.mybir as mybir
    for f in nc.m.functions:
        for bb in f.blocks:
            new_insts = []
            for ins in bb.instructions:
                si = ins.sync_info
                waits = list(si.on_wait) if (si and si.on_wait) else []
                if len(waits) > lim:
                    keep, extra = waits[:lim], waits[lim:]
                    for j in range(0, len(extra), lim):
                        nop = mybir.InstNoOp(
                            name=f"{ins.name}-ws{j}",
                            engine=ins.engine,
                            text_hint="waitsplit",
                            bass_nofuse=True,
                            sync_info=mybir.SyncInfo(
                                on_wait=list(extra[j:j + lim]), on_update=[]),
                        )
                        nc.register_instruction(nop, overwrite=True)
                        new_insts.append(nop)
                    si.on_wait.clear()
                    si.on_wait.extend(keep)
                new_insts.append(ins)
            bb.instructions.clear()
            bb.instructions.extend(new_insts)


# ---------------------------------------------------------------- program
def _build_program(debug=False):
    import contextlib

    import concourse.bass as bass
    import concourse.mybir as mybir
    import concourse.tile as tile

    _patch_tile_drain()

    f32 = mybir.dt.float32
    AF = mybir.ActivationFunctionType
    OP = mybir.AluOpType
    X = mybir.AxisListType.X

    nc = bass.Bass("TRN2")
    REGIONS.clear()

    def mark(label):
        REGIONS.append((label, nc.next_id()))

    def inp(name, shape):
        return nc.dram_tensor(name, shape, f32, kind="ExternalInput")

    hidown = inp("hidown", (128, H))
    hidhalo = inp("hidhalo", (3, H))
    wcat = inp("wcat", (H, 1536))
    # colpack: [ident(128)|tril(128)|triu(128)|poscol(4)|convw(24)|convb(6)|
    #           mysel(2)|c005+eps(2)] = 422 cols
    colpack = inp("colpack", (128, 422))
    # rowpack1 (phase-1 scoped): [bcat(1536)|ngrow(1024)]
    rowpack1 = inp("rowpack1", (1, 2560))
    # rowpack2 (persistent): [gbrow(1024)|maskvec(8)|mypos(128)]
    rowpack2 = inp("rowpack2", (1, 1160))
    stpack = inp("stpack", (128, 1536))   # [w1t|w2|w2t] in (p, dt, o) layout
    wot = inp("wot", (D, H))
    gwt = inp("gwt", (H, H))

    outp = nc.dram_tensor("outp", (128, H), f32, kind="ExternalOutput")
    dbg = {}
    if debug:
        def dbgout(name, shape):
            dbg[name] = nc.dram_tensor(name, shape, f32, kind="ExternalOutput")
            return dbg[name]
        dbg_kvq = dbgout("dbg_kvq", (128, 3, 2, 128))
        dbg_gates = dbgout("dbg_gates", (128, 3))
        dbg_cs = dbgout("dbg_cs", (128, 4, 2))
        dbg_dzpp = dbgout("dbg_dzpp", (128, 4, 256, 2))
        dbg_rpp = dbgout("dbg_rpp", (128, 4, 256, 2))
        dbg_st = dbgout("dbg_st", (128, 2, 256, 6))
        dbg_y2 = dbgout("dbg_y2", (128, 256, 2))
        dbg_ret = dbgout("dbg_ret", (128, 2, 128))

    agin = nc.dram_tensor("agin", (128, 515), f32, kind="Internal")
    agout = nc.dram_tensor("agout", (128 * N_CORES, 515), f32, kind="Internal",
                           addr_space="Shared")

    with tile.TileContext(nc) as tc:
        ctx = contextlib.ExitStack()
        with ctx:
            P_const = ctx.enter_context(tc.tile_pool(name="constp", bufs=1))
            P_keep = ctx.enter_context(tc.tile_pool(name="keepp", bufs=1))
            P_cols = ctx.enter_context(tc.tile_pool(name="colsp", bufs=2))
            P_rows = ctx.enter_context(tc.tile_pool(name="rowsp", bufs=1))
            PS_A = ctx.enter_context(tc.tile_pool(name="psa", bufs=5,
                                                  space="PSUM"))
            PS_B = ctx.enter_context(tc.tile_pool(name="psb", bufs=1,
                                                  space="PSUM"))

            MM = nc.tensor.matmul
            ACT = nc.scalar.activation
            V = nc.vector

            def psA(p, name):
                # all PS_A tiles share one 2KB/partition slot set
                return PS_A.tile([p, 512], f32, tag="A", name=name)

            def psB(shape, name):
                return PS_B.tile(shape, f32, tag="B", name=name)

            def cT(pool, shape, tag, bufs=None):
                return pool.tile(shape, f32, tag=tag, name=tag, bufs=bufs)

            # ---------------- constants ----------------
            ones2 = cT(P_const, [128, 128], "ones2")
            V.memset(ones2, 1.0)
            ones_row512 = cT(P_const, [1, T], "onesr")
            V.memset(ones_row512, 1.0)
            sb_cp = cT(P_const, [128, 422], "colpack")
            nc.sync.dma_start(sb_cp, colpack[:])
            sb_ident = sb_cp[:, 0:128]
            sb_tril = sb_cp[:, 128:256]
            sb_triu = sb_cp[:, 256:384]
            sb_poscol = sb_cp[:, 384:388]
            sb_convw = sb_cp[:, 388:412]
            sb_convb = sb_cp[:, 412:418]
            sb_mysel = sb_cp[:, 418:420]
            cst_005 = sb_cp[:, 420:421]
            cst_eps = sb_cp[:, 421:422]
            sb_rp2 = cT(P_const, [1, 1160], "rowpack2")
            nc.sync.dma_start(sb_rp2, rowpack2[:])
            sb_gbrow = sb_rp2[:, 0:1024]
            sb_maskvec = sb_rp2[:, 1024:1032]
            sb_mypos = sb_rp2[:, 1032:1160]

            sb_wot = cT(P_keep, [128, 2, H], "wot")
            for dt in range(2):
                nc.scalar.dma_start(sb_wot[:, dt, :],
                                    wot[128 * dt:128 * (dt + 1), :])
            sb_gwt = cT(P_keep, [128, 8, H], "gwt")
            for kb in range(8):
                nc.scalar.dma_start(sb_gwt[:, kb, :],
                                    gwt[128 * kb:128 * (kb + 1), :])

            st = {}
            sb_st0 = cT(P_keep, [128, 1536], "st0")
            nc.scalar.dma_start(sb_st0, stpack[:])
            st3 = sb_st0.rearrange("p (s a o) -> p s a o", s=3, a=2)
            st["w1t"] = st3[:, 0]
            st["w2"] = st3[:, 1]
            st["w2t"] = st3[:, 2]
            for nm in ("s1t", "s2", "s2t"):
                t_ = cT(P_keep, [128, 2, D], f"st_{nm}")
                V.memset(t_, 0.0)
                st[nm] = t_

            ng_bc = cT(P_keep, [128, H], "ngbc")

            # long-lived phase-1 products
            hidT = cT(P_keep, [128, 8, 128], "hidT")
            gates_own = cT(P_keep, [128, 3], "gates_own")
            kvq_own = cT(P_keep, [128, 3, 2, 128], "kvq_own")

            mark("setup")
            # ======================= PHASE 1 =======================
            with tc.tile_pool(name="ph1", bufs=1) as P1, \
                 tc.tile_pool(name="ph1w", bufs=2) as P1w:
                sb_rp1 = cT(P1, [1, 2560], "rowpack1")
                nc.sync.dma_start(sb_rp1, rowpack1[:])
                sb_bcat = sb_rp1[:, 0:1536]
                sb_ngrow = sb_rp1[:, 1536:2560]
                for half in range(2):
                    ps = psA(128, "ngbc_ps")
                    MM(ps, ones2[0:1, :],
                       sb_ngrow[:, half * 512:(half + 1) * 512],
                       start=True, stop=True)
                    nc.scalar.copy(ng_bc[:, half * 512:(half + 1) * 512], ps)
                sb_hown = cT(P1, [128, H], "hown")
                nc.sync.dma_start(sb_hown, hidown[:])
                sb_hhalo = cT(P1, [3, H], "hhalo")
                nc.sync.dma_start(sb_hhalo, hidhalo[:])

                for kb in range(8):
                    ps = psA(128, "tp")
                    nc.tensor.transpose(ps[:, 0:128],
                                        sb_hown[:, kb * 128:(kb + 1) * 128],
                                        sb_ident)
                    nc.scalar.copy(hidT[:, kb, :], ps[:, 0:128])
                hidTh = cT(P1, [128, 8, 3], "hidTh")
                for kb in range(8):
                    ps = psA(128, "tph")
                    nc.tensor.transpose(ps[:, 0:3],
                                        sb_hhalo[0:3, kb * 128:(kb + 1) * 128],
                                        sb_ident[0:3, 0:3])
                    nc.scalar.copy(hidTh[:, kb, :], ps[:, 0:3])

                # own projections (token-major), split into 3 psum tiles;
                # weight k-tiles stream through a rotating buffer
                psP = [psA(128, f"psP{ns}") for ns in range(3)]
                psH = [psA(3, "psH0"), psA(3, "psH1")]
                for kb in range(8):
                    wt = P1w.tile([128, 1536], f32, tag="wk", name="wk", bufs=3)
                    eng = nc.sync if kb % 2 == 0 else nc.scalar
                    eng.dma_start(wt, wcat[128 * kb:128 * (kb + 1), :])
                    for ns in range(3):
                        MM(psP[ns], hidT[:, kb, :],
                           wt[:, ns * 512:(ns + 1) * 512],
                           start=(kb == 0), stop=False)
                    MM(psH[0], hidTh[:, kb, :], wt[:, 0:512],
                       start=(kb == 0), stop=(kb == 7))
                    MM(psH[1][:, 0:256], hidTh[:, kb, :], wt[:, 512:768],
                       start=(kb == 0), stop=(kb == 7))
                for ns in range(3):
                    MM(psP[ns], ones2[0:1, :],
                       sb_bcat[:, ns * 512:(ns + 1) * 512],
                       start=False, stop=True)

                # gates from own proj
                scr1 = cT(P1w, [128, 256], "scr1")
                acc_a = cT(P_cols, [128, 1], "acc_a")
                ACT(scr1, psP[1][:, 256:512], AF.Sigmoid, accum_out=acc_a)
                scr2 = cT(P1w, [128, 256], "scr2")
                acc_e = cT(P_cols, [128, 1], "acc_e")
                ACT(scr2, psP[2][:, 0:256], AF.Sigmoid, accum_out=acc_e)
                scr3 = cT(P1w, [128, 256], "scr3")
                acc_t = cT(P_cols, [128, 1], "acc_t")
                # softplus(x) = -ln(sigmoid(-x)); accumulate the ln
                ACT(scr3, psP[2][:, 256:512], AF.Sigmoid, scale=-1.0)
                scr4 = cT(P1w, [128, 256], "scr4")
                ACT(scr4, scr3, AF.Ln, accum_out=acc_t)
                ACT(gates_own[:, 0:1], acc_e, AF.Ln, bias=cst_005[:, 0:1], scale=0.9 / 256.0)
                ACT(gates_own[:, 1:2], acc_a, AF.Ln, bias=1.0, scale=-0.1 / 256.0)
                nc.scalar.mul(gates_own[:, 2:3], acc_t, -LR / 256.0)

                pP1 = cT(P1, [128, 768], "pP1")
                V.tensor_copy(pP1[:, 0:512], psP[0])
                V.tensor_copy(pP1[:, 512:768], psP[1][:, 0:256])
                pH1 = cT(P1, [3, 768], "pH1")
                V.tensor_copy(pH1[:, 0:512], psH[0])
                V.tensor_copy(pH1[:, 512:768], psH[1][:, 0:256])

                xfull = cT(P1, [128, 3, 2, 131], "xfull")
                for pj in range(3):
                    for dt in range(2):
                        c0 = pj * 256 + dt * 128
                        ps = psA(128, "tp")
                        nc.tensor.transpose(ps[:, 0:128], pP1[:, c0:c0 + 128],
                                            sb_ident)
                        nc.scalar.copy(xfull[:, pj, dt, 3:131], ps[:, 0:128])
                        ps2 = psA(128, "tph")
                        nc.tensor.transpose(ps2[:, 0:3], pH1[0:3, c0:c0 + 128],
                                            sb_ident[0:3, 0:3])
                        nc.scalar.copy(xfull[:, pj, dt, 0:3], ps2[:, 0:3])

                for pj in range(3):
                    for dt in range(2):
                        x = xfull[:, pj, dt, :]
                        wb = dt * 12 + pj * 4
                        a0 = cT(P1w, [128, 128], "cv0")
                        V.tensor_scalar_mul(a0, x[:, 0:128],
                                            sb_convw[:, wb:wb + 1])
                        a1 = cT(P1w, [128, 128], "cv1")
                        V.scalar_tensor_tensor(a1, x[:, 1:129],
                                               sb_convw[:, wb + 1:wb + 2],
                                               a0, OP.mult, OP.add)
                        a2 = cT(P1w, [128, 128], "cv2")
                        V.scalar_tensor_tensor(a2, x[:, 2:130],
                                               sb_convw[:, wb + 2:wb + 3],
                                               a1, OP.mult, OP.add)
                        a3 = cT(P1w, [128, 128], "cv3")
                        V.scalar_tensor_tensor(a3, x[:, 3:131],
                                               sb_convw[:, wb + 3:wb + 4],
                                               a2, OP.mult, OP.add)
                        V.tensor_scalar_add(kvq_own[:, pj, dt, :], a3,
                                            sb_convb[:, dt * 3 + pj:
                                                     dt * 3 + pj + 1])

                if debug:
                    for pj in range(3):
                        nc.sync.dma_start(dbg_kvq[:, pj], kvq_own[:, pj])
                    nc.sync.dma_start(dbg_gates[:], gates_own[:])

                nc.sync.dma_start(
                    agin[:, 0:512],
                    kvq_own[:, 0:2, :, :].rearrange("p a b t -> p (a b t)"))
                nc.sync.dma_start(agin[:, 512:515], gates_own[:])

            mark("phase1")
            # phase-3 gate (hidden @ gW.T + gb) — independent of the
            # collective; emitted here so it fills the AG wait.
            gate_sb = cT(P_keep, [128, H], "gate_sb")
            gate_ps = psB([128, H], "gate_ps")
            for half in range(2):
                for kb in range(8):
                    MM(gate_ps[:, half * 512:(half + 1) * 512], hidT[:, kb, :],
                       sb_gwt[:, kb, half * 512:(half + 1) * 512],
                       start=(kb == 0), stop=False)
                MM(gate_ps[:, half * 512:(half + 1) * 512], ones2[0:1, :],
                   sb_gbrow[:, half * 512:(half + 1) * 512],
                   start=False, stop=True)
            V.tensor_copy(gate_sb, gate_ps)

            # ======================= ALLGATHER =======================
            nc.gpsimd.collective_compute(
                "AllGather", mybir.AluOpType.bypass,
                ins=[agin[:]], outs=[agout[:]],
                replica_groups=[list(range(N_CORES))],
            )

            K_fm = [cT(P_keep, [128, 2, T], f"kfm{cc}") for cc in range(2)]
            V_fm = [cT(P_keep, [128, 2, T], "vfm") for cc in range(2)]
            ag4 = agout.rearrange("(rk p) n -> p rk n", p=128)  # (128, 8, 515)
            for cc in range(2):
                for dt in range(2):
                    nc.sync.dma_start(
                        K_fm[cc][:, dt, :].rearrange("p (rr t) -> p rr t", rr=4),
                        ag4[:, 4 * cc:4 * cc + 4, dt * 128:(dt + 1) * 128])
                    nc.sync.dma_start(
                        V_fm[cc][:, dt, :].rearrange("p (rr t) -> p rr t", rr=4),
                        ag4[:, 4 * cc:4 * cc + 4, 256 + dt * 128:256 + (dt + 1) * 128])
            gates_all = cT(P_keep, [128, 8, 3], "gates_all")
            nc.sync.dma_start(gates_all, ag4[:, :, 512:515])

            # helper: (1,128) row slice -> (128,1) col
            def row_to_col(row_ap, out_col):
                ps = psA(128, "r2c")
                MM(ps[:, 0:1], row_ap, sb_ident[0:1, 0:1], start=True, stop=True)
                nc.scalar.copy(out_col, ps[:, 0:1])

            # ---------- local row machinery (chunk-independent) ----------
            bs_ps = psA(1, "bs")
            MM(bs_ps[:, 0:8], ones2[:, 0:1], gates_all[:, :, 1:2],
               start=True, stop=True)
            pre_mul = cT(P_cols, [1, 8], "pre_mul")
            V.tensor_tensor(pre_mul, bs_ps[:, 0:8], sb_maskvec, OP.mult)
            prefix = cT(P_cols, [1, 1], "prefix")
            V.tensor_reduce(prefix, pre_mul, X, OP.add)

            l1m_row = cT(P_cols, [1, 128], "l1m_row")
            psx = psA(1, "l1mr")
            MM(psx[:, 0:128], gates_own[:, 1:2], sb_ident, start=True, stop=True)
            nc.scalar.copy(l1m_row, psx[:, 0:128])
            la_my_row = cT(P_cols, [1, 128], "la_my_row")
            V.tensor_tensor_scan(la_my_row, ones_row512[:, 0:128], l1m_row,
                                 prefix, OP.mult, OP.add)
            la_my_col = cT(P_cols, [128, 1], "la_my_col")
            row_to_col(la_my_row, la_my_col)
            a_col_my = cT(P_cols, [128, 1], "a_col_my")
            ACT(a_col_my, la_my_col, AF.Exp)
            la_my_bc = cT(P_keep, [128, 128], "la_my_bc")
            psx = psA(128, "lamybc")
            MM(psx[:, 0:128], ones2[0:1, :], la_my_row, start=True, stop=True)
            nc.scalar.copy(la_my_bc, psx[:, 0:128])
            mypos_bc = cT(P_keep, [128, 128], "mypos_bc")
            psx = psA(128, "myposbc")
            MM(psx[:, 0:128], ones2[0:1, :], sb_mypos, start=True, stop=True)
            nc.scalar.copy(mypos_bc, psx[:, 0:128])

            mark("ag_unpack")
            with tc.tile_pool(name="workp", bufs=1) as P_work:
                ret = cT(P_keep, [128, 2, 128], "ret")
                for cc in range(2):
                    Kc, Vc = K_fm[cc], V_fm[cc]
                    th_cols = gates_all[:, 4 * cc:4 * cc + 4, 2:3].rearrange("p a o -> p (a o)")

                    # ---- chunk scalar rows/cols ----
                    g_rows = []
                    for gi in range(2):
                        ps = psA(4, f"gr{gi}")
                        MM(ps[:, 0:128],
                           gates_all[:, 4 * cc:4 * cc + 4, gi:gi + 1],
                           sb_ident, start=True, stop=True)
                        gsb = cT(P_cols, [4, 128], f"gsb{gi}")
                        nc.scalar.copy(gsb, ps[:, 0:128])
                        row = cT(P_cols, [1, T], f"graw{gi}")
                        for rr in range(4):
                            nc.sync.dma_start(row[:, rr * 128:(rr + 1) * 128],
                                              gsb[rr:rr + 1, :])
                        srow = cT(P_cols, [1, T], f"gcum{gi}")
                        V.tensor_tensor_scan(srow, ones_row512, row, 0.0,
                                             OP.mult, OP.add)
                        g_rows.append(srow)
                    le_row, la_row = g_rows

                    le_cols = cT(P_cols, [128, 4], "le_cols")
                    la_cols = cT(P_cols, [128, 4], "la_cols")
                    for b in range(4):
                        row_to_col(le_row[:, b * 128:(b + 1) * 128],
                                   le_cols[:, b:b + 1])
                        row_to_col(la_row[:, b * 128:(b + 1) * 128],
                                   la_cols[:, b:b + 1])
                    E_cols = cT(P_cols, [128, 4], "E_cols")
                    ACT(E_cols, le_cols, AF.Exp)

                    le_bc = psA(128, "lebc")
                    MM(le_bc, ones2[0:1, :], le_row, start=True, stop=True)
                    G = cT(P_work, [128, 4, T], "G")
                    for b in range(4):
                        scrg = psA(128, "scrg")
                        V.tensor_scalar(scrg, le_bc, le_cols[:, b:b + 1], 0.0,
                                        OP.subtract, OP.max)
                        ACT(G[:, b, :], scrg, AF.Exp, scale=-1.0)
                        V.tensor_tensor(G[:, b, b * 128:(b + 1) * 128],
                                        G[:, b, b * 128:(b + 1) * 128],
                                        sb_tril, OP.mult)

                    # ---- grads at chunk-start state ----
                    w1t_c, w2_c, w2t_c = st["w1t"], st["w2"], st["w2t"]
                    s1t_c, s2t_c = st["s1t"], st["s2t"]

                    Z_fm = psB([128, 2, T], "Z_fm")
                    for m in range(2):
                        for kt in range(2):
                            MM(Z_fm[:, m, :],
                               w1t_c[:, kt, m * 128:(m + 1) * 128],
                               Kc[:, kt, :], start=(kt == 0), stop=(kt == 1))
                    Hp_fm = cT(P_work, [128, 2, T], "hpfm")
                    ACT(Hp_fm[:, 0, :], Z_fm[:, 0, :], AF.Silu)
                    ACT(Hp_fm[:, 1, :], Z_fm[:, 1, :], AF.Silu)

                    Z_tm = [psA(128, "ztm0"), psA(128, "ztm1")]
                    for b in range(4):
                        for kt in range(2):
                            MM(Z_tm[b // 2][:, (b % 2) * 256:(b % 2 + 1) * 256],
                               Kc[:, kt, b * 128:(b + 1) * 128],
                               w1t_c[:, kt, :], start=(kt == 0), stop=(kt == 1))
                    Hp_tm = cT(P_work, [128, 4, D], "hptm")
                    sp_tm = cT(P_work, [128, 4, D], "sptm")
                    for b in range(4):
                        zt = Z_tm[b // 2][:, (b % 2) * 256:(b % 2 + 1) * 256]
                        ACT(Hp_tm[:, b, :], zt, AF.Silu)
                    for b in range(4):
                        zt = Z_tm[b // 2][:, (b % 2) * 256:(b % 2 + 1) * 256]
                        ACT(sp_tm[:, b, :], zt, AF.Derivative_silu)

                    Pred = psB([128, 2, T], "Pred")
                    for m in range(2):
                        for kt in range(2):
                            MM(Pred[:, m, :],
                               w2t_c[:, kt, m * 128:(m + 1) * 128],
                               Hp_fm[:, kt, :], start=(kt == 0), stop=(kt == 1))
                    PV_fm = cT(P_work, [128, 2, T], "pvfm")
                    V.tensor_sub(PV_fm[:, 0, :], Pred[:, 0, :], Vc[:, 0, :])
                    V.tensor_sub(PV_fm[:, 1, :], Pred[:, 1, :], Vc[:, 1, :])

                    dH_tm = [psA(128, "dhtm0"), psA(128, "dhtm1")]
                    for b in range(4):
                        for kt in range(2):
                            MM(dH_tm[b // 2][:, (b % 2) * 256:(b % 2 + 1) * 256],
                               PV_fm[:, kt, b * 128:(b + 1) * 128],
                               w2_c[:, kt, :], start=(kt == 0), stop=(kt == 1))
                    dzpp = cT(P_work, [128, 4, D], "dzpp")
                    nzh_cols = cT(P_cols, [128, 4], "nzh_cols")
                    for b in range(4):
                        V.tensor_tensor(
                            dzpp[:, b, :],
                            dH_tm[b // 2][:, (b % 2) * 256:(b % 2 + 1) * 256],
                            sp_tm[:, b, :], OP.mult)
                        scrD = psA(128, "scrD")
                        V.scalar_tensor_tensor(scrD[:, 0:256], dzpp[:, b, :],
                                               1.0, dzpp[:, b, :],
                                               OP.bypass, OP.mult,
                                               accum_out=nzh_cols[:, b:b + 1])

                    ncols = {}
                    for nm, src in (("nk", Kc), ("npv", PV_fm), ("nh", Hp_fm)):
                        psn = psA(1, f"nrow_{nm}")
                        for kt in range(2):
                            sq = cT(P_work, [128, T], "sq")
                            V.tensor_tensor(sq, src[:, kt, :], src[:, kt, :],
                                            OP.mult)
                            MM(psn[:, 0:512], ones2[:, 0:1], sq,
                               start=(kt == 0), stop=(kt == 1))
                        nr = cT(P_cols, [1, T], f"nr_{nm}")
                        nc.scalar.copy(nr, psn[:, 0:512])
                        cl = cT(P_cols, [128, 4], f"nc_{nm}")
                        for b in range(4):
                            row_to_col(nr[:, b * 128:(b + 1) * 128],
                                       cl[:, b:b + 1])
                        ncols[nm] = cl

                    cs_cols = cT(P_cols, [128, 4], "cs_cols")
                    g1 = cT(P_cols, [128, 4], "g1c")
                    V.tensor_tensor(g1, nzh_cols, ncols["nk"], OP.mult)
                    g2 = cT(P_cols, [128, 4], "g2c")
                    V.tensor_tensor(g2, ncols["npv"], ncols["nh"], OP.mult)
                    V.tensor_add(g1, g1, g2)
                    ACT(g1, g1, AF.Sqrt, scale=4.0 * GS * GS)
                    V.tensor_scalar_max(g1, g1, EPS)
                    V.reciprocal(g1, g1)
                    V.tensor_scalar_min(g1, g1, 1.0)
                    V.tensor_tensor(g1, g1, th_cols, OP.mult)
                    V.tensor_scalar_mul(cs_cols, g1, 2.0 * GS)
                    if debug:
                        nc.sync.dma_start(dbg_cs[:, :, cc], cs_cols[:])

                    for b in range(4):
                        V.tensor_scalar_mul(dzpp[:, b, :], dzpp[:, b, :],
                                            cs_cols[:, b:b + 1])
                    rpp = cT(P_work, [128, 4, D], "rpp")
                    for b in range(4):
                        for dt in range(2):
                            ps = psA(128, "tp")
                            nc.tensor.transpose(
                                ps[:, 0:128],
                                PV_fm[:, dt, b * 128:(b + 1) * 128], sb_ident)
                            V.tensor_scalar_mul(
                                rpp[:, b, dt * 128:(dt + 1) * 128],
                                ps[:, 0:128], cs_cols[:, b:b + 1])
                    if debug:
                        for b in range(4):
                            nc.sync.dma_start(dbg_dzpp[:, b, :, cc],
                                              dzpp[:, b, :])
                            nc.sync.dma_start(dbg_rpp[:, b, :, cc],
                                              rpp[:, b, :])

                    # ---- state update (chunk 0 only) ----
                    if cc == 0:
                        leT = le_row[:, T - 1:T]
                        laT = la_row[:, T - 1:T]
                        gse_row = cT(P_rows, [1, T], "gse_row")
                        V.tensor_scalar_sub(gse_row, le_row, leT)
                        ACT(gse_row, gse_row, AF.Exp, scale=-1.0)
                        cwL_row = cT(P_rows, [1, T], "cwL_row")
                        V.tensor_scalar_sub(cwL_row, la_row, laT)
                        ACT(cwL_row, cwL_row, AF.Exp, scale=-1.0)
                        gse_cols = cT(P_cols, [128, 4], "gse_cols")
                        cwL_cols = cT(P_cols, [128, 4], "cwL_cols")
                        for b in range(4):
                            row_to_col(gse_row[:, b * 128:(b + 1) * 128],
                                       gse_cols[:, b:b + 1])
                            row_to_col(cwL_row[:, b * 128:(b + 1) * 128],
                                       cwL_cols[:, b:b + 1])

                        cw_cols = cT(P_cols, [128, 4], "cw_cols")
                        for a in range(4):
                            ps = psA(128, "cw")
                            for b in range(a, 4):
                                MM(ps[:, 0:1], G[:, b, a * 128:(a + 1) * 128],
                                   cwL_cols[:, b:b + 1],
                                   start=(b == a), stop=(b == 3))
                            nc.scalar.copy(cw_cols[:, a:a + 1], ps[:, 0:1])

                        tmp14 = cT(P_cols, [128, 4], "tmp14")
                        V.tensor_tensor(tmp14, cwL_cols, E_cols, OP.mult)
                        psx = psA(1, "bt")
                        MM(psx[:, 0:4], ones2[:, 0:1], tmp14,
                           start=True, stop=True)
                        sc3 = cT(P_cols, [1, 3], "sc3")
                        V.tensor_reduce(sc3[:, 2:3], psx[:, 0:4], X, OP.add)
                        ACT(sc3[:, 0:1], leT, AF.Exp)
                        ACT(sc3[:, 1:2], laT, AF.Exp)
                        psx = psA(128, "sccols")
                        MM(psx[:, 0:3], ones2[0:1, :], sc3, start=True, stop=True)
                        sc_cols = cT(P_cols, [128, 3], "sc_cols")
                        nc.scalar.copy(sc_cols, psx[:, 0:3])
                        ET_col = sc_cols[:, 0:1]
                        AT_col = sc_cols[:, 1:2]
                        bT_col = sc_cols[:, 2:3]

                        K_S = cT(P_work, [128, 4, D], "K_S")
                        K_C = cT(P_work, [128, 4, D], "K_C")
                        for b in range(4):
                            for dt in range(2):
                                ps = psA(128, "tp")
                                nc.tensor.transpose(
                                    ps[:, 0:128],
                                    Kc[:, dt, b * 128:(b + 1) * 128], sb_ident)
                                V.tensor_scalar_mul(
                                    K_S[:, b, dt * 128:(dt + 1) * 128],
                                    ps[:, 0:128], gse_cols[:, b:b + 1])
                                V.tensor_scalar_mul(
                                    K_C[:, b, dt * 128:(dt + 1) * 128],
                                    ps[:, 0:128], cw_cols[:, b:b + 1])
                        Hp_S = cT(P_work, [128, 4, D], "Hp_S")
                        Hp_C = cT(P_work, [128, 4, D], "Hp_C")
                        for b in range(4):
                            V.tensor_scalar_mul(Hp_S[:, b, :], Hp_tm[:, b, :],
                                                gse_cols[:, b:b + 1])
                            V.tensor_scalar_mul(Hp_C[:, b, :], Hp_tm[:, b, :],
                                                cw_cols[:, b:b + 1])

                        new_st = {}
                        for nm, lh, rh, snm in (
                            ("s1t", K_S, dzpp, None), ("w1t", K_C, dzpp, "s1t"),
                            ("s2t", Hp_S, rpp, None), ("w2t", Hp_C, rpp, "s2t"),
                        ):
                            pu = psA(128, f"upd_{nm}")
                            # accumulate both halves (m=0,1) into one 2KB slot
                            for m in range(2):
                                for b in range(4):
                                    MM(pu[:, m * 256:(m + 1) * 256],
                                       lh[:, b, m * 128:(m + 1) * 128],
                                       rh[:, b, :],
                                       start=(b == 0), stop=(b == 3))
                            t_ = cT(P_keep, [128, 2, D], f"nst_{nm}")
                            tflat = t_.rearrange("p a o -> p (a o)")
                            oflat = st[nm].rearrange("p a o -> p (a o)")
                            tmp = cT(P_work, [128, 2 * D], "sttmp")
                            if snm is None:
                                V.tensor_scalar_mul(tmp, oflat, ET_col)
                            else:
                                V.tensor_scalar_mul(tmp, oflat, AT_col)
                                V.scalar_tensor_tensor(
                                    tmp, st[snm].rearrange("p a o -> p (a o)"),
                                    bT_col, tmp, OP.mult, OP.add)
                            V.tensor_sub(tflat, tmp, pu[:, 0:512])
                            new_st[nm] = t_
                        for nm, snm in (("s2", "s2t"), ("w2", "w2t")):
                            t_ = cT(P_keep, [128, 2, D], f"nst_{nm}")
                            for m in range(2):
                                for dt in range(2):
                                    ps = psA(128, "tp")
                                    nc.tensor.transpose(
                                        ps[:, 0:128],
                                        new_st[snm][:, dt,
                                                    m * 128:(m + 1) * 128],
                                        sb_ident)
                                    V.tensor_copy(
                                        t_[:, m, dt * 128:(dt + 1) * 128],
                                        ps[:, 0:128])
                            new_st[nm] = t_
                        if debug:
                            for si, nm in enumerate(("w1t", "s1t", "w2", "w2t",
                                                     "s2", "s2t")):
                                for dt in range(2):
                                    nc.sync.dma_start(dbg_st[:, dt, :, si],
                                                      new_st[nm][:, dt, :])

                    mark(f"cols{cc}")
                    # ---- retrieval for my 128 rows ----
                    L_my = cT(P_work, [128, 4, 128], "lmy")
                    for b in range(4):
                        scrm = psA(128, "scrm")
                        V.tensor_scalar(scrm[:, 0:128], la_my_bc,
                                        la_cols[:, b:b + 1],
                                        0.0, OP.subtract, OP.min)
                        ACT(scrm[:, 0:128], scrm[:, 0:128], AF.Exp)
                        scrm2 = cT(P_work, [128, 128], "scrm2")
                        V.tensor_scalar(scrm2, mypos_bc, sb_poscol[:, b:b + 1],
                                        0.0, OP.subtract, OP.is_ge)
                        V.tensor_tensor(L_my[:, b, :], scrm[:, 0:128], scrm2,
                                        OP.mult)

                    psb_ = psA(1, "brow")
                    for b in range(4):
                        tmpL = cT(P_work, [128, 128], "tmpL")
                        V.tensor_scalar_mul(tmpL, L_my[:, b, :],
                                            E_cols[:, b:b + 1])
                        MM(psb_[:, 0:128], ones2[:, 0:1], tmpL,
                           start=(b == 0), stop=(b == 3))
                    b_row = cT(P_cols, [1, 128], "b_row")
                    nc.scalar.copy(b_row, psb_[:, 0:128])
                    b_col_my = cT(P_cols, [128, 1], "b_col_my")
                    row_to_col(b_row, b_col_my)

                    q_own = kvq_own[:, 2, :, :]

                    QK = psA(128, "qk")
                    for a in range(4):
                        for dt in range(2):
                            MM(QK[:, a * 128:(a + 1) * 128],
                               Kc[:, dt, a * 128:(a + 1) * 128],
                               q_own[:, dt, :], start=(dt == 0), stop=(dt == 1))
                    CTp = psA(128, "ct")
                    for a in range(4):
                        for b in range(a, 4):
                            MM(CTp[:, a * 128:(a + 1) * 128],
                               G[:, b, a * 128:(a + 1) * 128],
                               L_my[:, b, :], start=(b == a), stop=(b == 3))
                    CT_sb = cT(P_work, [128, 4, 128], "ctsb")
                    V.tensor_copy(CT_sb.rearrange("p a t -> p (a t)"), CTp)
                    CQK = cT(P_work, [128, 4, 128], "cqk")
                    V.tensor_tensor(CQK.rearrange("p a t -> p (a t)"),
                                    CT_sb.rearrange("p a t -> p (a t)"),
                                    QK, OP.mult)

                    U1 = psA(128, "u1")
                    for a in range(4):
                        MM(U1[:, 0:256], CQK[:, a, :], dzpp[:, a, :],
                           start=(a == 0), stop=(a == 3))
                    X1 = psA(128, "x1")
                    for dt in range(2):
                        MM(X1[:, 0:256], q_own[:, dt, :], w1t_c[:, dt, :],
                           start=(dt == 0), stop=(dt == 1))
                    X2 = psA(128, "x2")
                    for dt in range(2):
                        MM(X2[:, 0:256], q_own[:, dt, :], s1t_c[:, dt, :],
                           start=(dt == 0), stop=(dt == 1))
                    u1sb = cT(P_work, [128, D], "usb")
                    V.tensor_copy(u1sb, U1[:, 0:256])
                    y1 = cT(P_work, [128, D], "ysb")
                    V.scalar_tensor_tensor(y1, X2[:, 0:256], b_col_my, u1sb,
                                           OP.mult, OP.subtract)
                    V.scalar_tensor_tensor(y1, X1[:, 0:256], a_col_my, y1,
                                           OP.mult, OP.add)
                    H_tm = cT(P_work, [128, D], "htm")
                    ACT(H_tm, y1, AF.Silu)
                    H_fm = cT(P_work, [128, 2, 128], "hfm")
                    for dt in range(2):
                        ps = psA(128, "tp")
                        nc.tensor.transpose(ps[:, 0:128],
                                            H_tm[:, dt * 128:(dt + 1) * 128],
                                            sb_ident)
                        nc.scalar.copy(H_fm[:, dt, :], ps[:, 0:128])

                    HHp = psA(128, "qk2")
                    for a in range(4):
                        for dt in range(2):
                            MM(HHp[:, a * 128:(a + 1) * 128],
                               Hp_fm[:, dt, a * 128:(a + 1) * 128],
                               H_fm[:, dt, :], start=(dt == 0), stop=(dt == 1))
                    CHHp = cT(P_work, [128, 4, 128], "chhp")
                    V.tensor_tensor(CHHp.rearrange("p a t -> p (a t)"),
                                    CT_sb.rearrange("p a t -> p (a t)"),
                                    HHp, OP.mult)

                    U2 = psA(128, "u2")
                    for a in range(4):
                        MM(U2[:, 0:256], CHHp[:, a, :], rpp[:, a, :],
                           start=(a == 0), stop=(a == 3))
                    X3 = psA(128, "x3")
                    for dt in range(2):
                        MM(X3[:, 0:256], H_fm[:, dt, :], w2t_c[:, dt, :],
                           start=(dt == 0), stop=(dt == 1))
                    X4 = psA(128, "x4")
                    for dt in range(2):
                        MM(X4[:, 0:256], H_fm[:, dt, :], s2t_c[:, dt, :],
                           start=(dt == 0), stop=(dt == 1))
                    u2sb = cT(P_work, [128, D], "usb")
                    V.tensor_copy(u2sb, U2[:, 0:256])
                    y2 = cT(P_work, [128, D], "ysb")
                    V.scalar_tensor_tensor(y2, X4[:, 0:256], b_col_my, u2sb,
                                           OP.mult, OP.subtract)
                    V.scalar_tensor_tensor(y2, X3[:, 0:256], a_col_my, y2,
                                           OP.mult, OP.add)
                    if debug:
                        nc.sync.dma_start(dbg_y2[:, :, cc], y2[:])

                    for dt in range(2):
                        ps = psA(128, "tp")
                        nc.tensor.transpose(ps[:, 0:128],
                                            y2[:, dt * 128:(dt + 1) * 128],
                                            sb_ident)
                        if cc == 0:
                            V.tensor_scalar_mul(ret[:, dt, :], ps[:, 0:128],
                                                sb_mysel[:, 0:1])
                        else:
                            V.scalar_tensor_tensor(ret[:, dt, :], ps[:, 0:128],
                                                   sb_mysel[:, 1:2],
                                                   ret[:, dt, :],
                                                   OP.mult, OP.add)

                    if cc == 0:
                        st = new_st
                        mark("rows0")

                if debug:
                    nc.sync.dma_start(dbg_ret[:], ret[:])

                mark("rows1")
                # ======================= PHASE 3 =======================
                po = psB([128, H], "po")
                for half in range(2):
                    for dt in range(2):
                        MM(po[:, half * 512:(half + 1) * 512], ret[:, dt, :],
                           sb_wot[:, dt, half * 512:(half + 1) * 512],
                           start=(dt == 0), stop=(dt == 1))
                ssq = cT(P_cols, [128, 1], "ssq")
                ss2 = cT(P_cols, [128, 1], "ss2")
                sqh = cT(P_work, [128, T], "sq")
                ACT(sqh, po[:, 0:512], AF.Square, accum_out=ssq)
                sqh2 = cT(P_work, [128, T], "sq")
                ACT(sqh2, po[:, 512:1024], AF.Square, accum_out=ss2)
                V.tensor_add(ssq, ssq, ss2)
                rstd = cT(P_cols, [128, 1], "rstd")
                ACT(rstd, ssq, AF.Sqrt, bias=cst_eps[:, 0:1], scale=1.0 / H)
                V.reciprocal(rstd, rstd)
                normed = cT(P_work, [128, H], "sq")
                V.tensor_scalar_mul(normed, po, rstd)
                V.tensor_tensor(normed, normed, ng_bc, OP.mult)

                V.tensor_tensor(normed, normed, gate_sb, OP.mult)
                nc.sync.dma_start(outp[:], normed)

    _split_excess_waits(nc, lim=1)
    return nc


# ---------------------------------------------------------------- host side
def _prep_inputs(inputs):
    ii = {k: np.ascontiguousarray(np.asarray(v), dtype=np.float32)
          for k, v in inputs.items()}
    hidden = ii["hidden"].reshape(S, H)

    wcat = np.concatenate([ii["Wk"].T, ii["Wv"].T, ii["Wq"].T,
                           ii["aW"].T, ii["eW"].T, ii["tW"].T], axis=1)
    bcat = np.concatenate([np.zeros(768, np.float32),
                           ii["ab"], ii["eb"], ii["tb"]])[None]
    convw = np.zeros((128, 24), np.float32)
    convb = np.zeros((128, 6), np.float32)
    for dt in range(2):
        for pj, (w, b) in enumerate(((ii["ckw"], ii["ckb"]),
                                     (ii["cvw"], ii["cvb"]),
                                     (ii["cqw"], ii["cqb"]))):
            convw[:, dt * 12 + pj * 4:dt * 12 + pj * 4 + 4] = \
                w[dt * 128:(dt + 1) * 128]
            convb[:, dt * 3 + pj] = b[dt * 128:(dt + 1) * 128]

    poscol = (np.arange(128, dtype=np.float32)[:, None]
              + 128.0 * np.arange(4, dtype=np.float32)[None, :])
    # stpack layout: (p, s, a, o): s in (w1t, w2, w2t); a = d-tile
    sp = np.zeros((128, 3, 2, 256), np.float32)
    for si, w in enumerate((ii["mW1"].T, ii["mW2"], ii["mW2"].T)):
        for a in range(2):
            sp[:, si, a, :] = w[128 * a:128 * (a + 1), :]
    stpack = sp.reshape(128, 1536)

    in_maps = []
    for c in range(N_CORES):
        g, r = c // 4, c % 4
        start = 128 * c
        halo = np.zeros((3, H), np.float32)
        if start >= 3:
            halo[:] = hidden[start - 3:start]
        colpack = np.zeros((128, 422), np.float32)
        colpack[:, 0:128] = np.eye(128, dtype=np.float32)
        colpack[:, 128:256] = np.tril(np.ones((128, 128), np.float32))
        colpack[:, 256:384] = np.triu(np.ones((128, 128), np.float32))
        colpack[:, 384:388] = poscol
        colpack[:, 388:412] = convw
        colpack[:, 412:418] = convb
        colpack[:, 418 + g] = 1.0
        colpack[:, 420] = 0.05
        colpack[:, 421] = EPS
        rowpack1 = np.zeros((1, 2560), np.float32)
        rowpack1[0, 0:1536] = bcat[0]
        rowpack1[0, 1536:2560] = ii["norm_g"]
        rowpack2 = np.zeros((1, 1160), np.float32)
        rowpack2[0, 0:1024] = ii["gb"]
        mv = np.zeros(8, np.float32)
        for j in range(N_CORES):
            if j // 4 == g and j % 4 < r:
                mv[j] = 1.0
        rowpack2[0, 1024:1032] = mv
        rowpack2[0, 1032:1160] = 128.0 * r + np.arange(128, dtype=np.float32)
        m = {
            "wcat": np.ascontiguousarray(wcat),
            "colpack": colpack,
            "rowpack1": rowpack1,
            "rowpack2": rowpack2,
            "stpack": np.ascontiguousarray(stpack),
            "wot": np.ascontiguousarray(ii["Wo"].T),
            "gwt": np.ascontiguousarray(ii["gW"].T),
            "hidown": np.ascontiguousarray(hidden[start:start + 128]),
            "hidhalo": halo,
        }
        in_maps.append(m)
    return in_maps


def kernel(**inputs):
    from concourse.bass_utils import run_bass_kernel_spmd

    debug = bool(inputs.pop("_debug", False))
    key = ("prog", debug)
    if key not in _CACHE:
        _CACHE[key] = _build_program(debug=debug)
    nc = _CACHE[key]

    in_maps = _prep_inputs(inputs)
    res = run_bass_kernel_spmd(nc, in_maps, core_ids=list(range(N_CORES)))
    out = np.concatenate([res.results[c]["outp"] for c in range(N_CORES)],
                         axis=0)[None]
    if debug:
        return out.astype(np.float32), res
    return out.astype(np.float32)


def analyze_cost():
    import re
    import concourse.mybir as mb
    from concourse.cost_model import InstructionCostModel
    from concourse.timeline_sim import TimelineSim
    from concourse.hw_specs import get_hw_spec

    nc = _CACHE.get(("prog", False)) or _build_program(debug=False)
    regions = list(REGIONS)
    rec = {}

    class RecModel(InstructionCostModel):
        def visit(self, instruction, sim):
            tls = super().visit(instruction, sim)
            if instruction.name not in rec:
                dur = 0.0
                for tl in tls:
                    for ev in tl:
                        s = str(ev)
                        m = re.match(r"Delay\(ns=([0-9.]+)\)", s)
                        if m:
                            dur += float(m.group(1))
                rec[instruction.name] = (str(instruction.engine),
                                         type(instruction).__name__, dur)
            return tls

    ts = TimelineSim(nc, no_exec=True, cost_model=RecModel(get_hw_spec("TRN2")))
    total = ts.simulate()

    def region_of(iid):
        for label, last in regions:
            if iid <= last:
                return label
        return "tail"

    agg = {}
    for nm, (eng, ty, dur) in rec.items():
        m = re.match(r"I-(\d+)", nm)
        iid = int(m.group(1)) if m else 10**9
        reg = region_of(iid)
        key = (reg, eng)
        agg[key] = agg.get(key, 0.0) + dur
    print(f"sim total: {total:.0f} ns")
    regs = [r for r, _ in regions] + ["tail"]
    engs = sorted({k[1] for k in agg})
    print(f"{'region':10s}" + "".join(f"{e.split('.')[-1]:>10s}" for e in engs))
    for r in regs:
        row = "".join(f"{agg.get((r, e), 0) / 1000:10.1f}" for e in engs)
        print(f"{r:10s}" + row)
    by_ty = {}
    for nm, (eng, ty, dur) in rec.items():
        by_ty[ty] = by_ty.get(ty, 0.0) + dur
    for ty, d in sorted(by_ty.items(), key=lambda x: -x[1])[:12]:
        print(f"  {ty:28s} {d/1000:9.1f} us")
    return total
